# revision 22
# baseline (speedup 1.0000x reference)
"""EuclideanCodebook (VQ) forward on 8 Trainium2 NeuronCores.

Data-parallel over the flattened N = batch*frames axis: each core gets an
8192-row slab of x and the full [1024, 128] codebook.

Host-side prep is layout-only (transposes for DMA/matmul efficiency); all
arithmetic runs on device:

  PE:  dist = 2*x·e^T (fp32 matmul into PSUM; the ||x||^2 term of the
       reference is row-constant -> argmax-invariant)
  DVE: adds -||e_k||^2 (broadcast tile built on device at setup) while
       copying PSUM -> SBUF, then Max8 + MaxIndex give the exact fp32
       argmax per row
  GPSIMD: indirect DMA gather embed[idx] -> quantize rows

Device layouts (per core): row n = t*128 + p for tile t in [0,64),
partition p in [0,128).  x arrives transposed ([128 d, 8192 n]) so every
tile's lhsT is a direct SBUF slice; q leaves as [128 p, 64 t, 128 d] and
the host permutes back to row order.
"""

import numpy as np

import concourse.bacc as bacc
import concourse.bass as bass
import concourse.mybir as mybir
import concourse.tile as tile
from concourse.bass_utils import run_bass_kernel_spmd

N_CORES = 8
N_FULL = 65536          # 8 * 8192 rows
N_LOC = N_FULL // N_CORES   # 8192 rows per core
K = 1024                # codebook size
D = 128                 # feature dim
P = 128                 # partitions
TILES = N_LOC // P      # 64 row-tiles per core
CHUNK_SIZES = (4, 12, 16, 16, 12, 4)   # tiles per x/q DMA chunk

FP32 = mybir.dt.float32
U32 = mybir.dt.uint32
I32 = mybir.dt.int32

_COMPILED = {}


def _build(reps=1):
    nc = bacc.Bacc(
        "TRN2", target_bir_lowering=False, debug=False, num_devices=N_CORES
    )
    xt_d = nc.dram_tensor("xt", [P, N_LOC], FP32, kind="ExternalInput")
    eT_d = nc.dram_tensor("eT", [P, K], FP32, kind="ExternalInput")
    e_d = nc.dram_tensor("e", [K, D], FP32, kind="ExternalInput")
    q_d = nc.dram_tensor("q", [P, TILES * D], FP32, kind="ExternalOutput")
    # one 8-wide uint32 slot per row-tile (MaxIndex writes all 8 lanes);
    # host keeps lane 0 of each slot.
    ind_d = nc.dram_tensor("ind", [P, TILES * 8], I32, kind="ExternalOutput")

    with tile.TileContext(nc) as tc:
        with (
            tc.tile_pool(name="const", bufs=1) as const_pool,
            tc.tile_pool(name="xin", bufs=3) as x_pool,
            tc.tile_pool(name="g", bufs=2) as g_pool,
            tc.tile_pool(name="mx8", bufs=2) as mx8_pool,
            tc.tile_pool(name="s", bufs=4) as s_pool,
            tc.tile_pool(name="pdist", bufs=4, space="PSUM") as pdist_pool,
        ):
            # ---- constants / setup ----------------------------------------
            negq = const_pool.tile([P, P], FP32)   # all -0.25
            nc.gpsimd.memset(negq[:], -0.25)
            idxbig = const_pool.tile([P, TILES * 8], U32)

            embT2 = const_pool.tile([P, K], FP32)  # [d, k] = 2*e[k, d]
            eTs = const_pool.tile([P, K], FP32)
            nc.scalar.dma_start(eTs[:], eT_d.ap())
            nc.vector.tensor_scalar_mul(embT2[:], eTs[:], 2.0)
            # sq = (2 e)^2 on ACT; e2negbc[m, k] = -0.25*sum_d sq = -||e_k||^2
            sq = const_pool.tile([P, K], FP32)
            nc.scalar.activation(
                sq[:], embT2[:], mybir.ActivationFunctionType.Square
            )
            e2bc_ps = pdist_pool.tile([P, K], FP32, space="PSUM", tag="pd")
            for h in range(2):
                sl = slice(h * 512, (h + 1) * 512)
                nc.tensor.matmul(
                    out=e2bc_ps[:, sl],
                    lhsT=negq[:],
                    rhs=sq[:, sl],
                    start=True,
                    stop=True,
                )
            e2negbc = const_pool.tile([P, K], FP32)
            nc.vector.tensor_copy(e2negbc[:], e2bc_ps[:])

            # ---- main loop (reps>1 only for steady-state timing probes) ---
            for _rep in range(reps):
                t0 = 0
                for tpc in CHUNK_SIZES:
                    xc = x_pool.tile([P, 16 * P], FP32, tag="xc")
                    nc.sync.dma_start(
                        xc[:, : tpc * P], xt_d.ap()[:, t0 * P : (t0 + tpc) * P]
                    )
                    gth = g_pool.tile([P, 16 * D], FP32, tag="gth")
                    for j in range(tpc):
                        t = t0 + j
                        dist_ps = pdist_pool.tile(
                            [P, K], FP32, space="PSUM", tag="pd"
                        )
                        # PE computes raw 2x·e; DVE folds in -||e_k||^2
                        # while copying to SBUF, then scans there.
                        for h in range(2):
                            sl = slice(h * 512, (h + 1) * 512)
                            nc.tensor.matmul(
                                out=dist_ps[:, sl],
                                lhsT=xc[:, j * P : (j + 1) * P],
                                rhs=embT2[:, sl],
                                start=True,
                                stop=True,
                            )
                        s = s_pool.tile([P, K], FP32, tag="s")
                        nc.vector.tensor_add(s[:], dist_ps[:], e2negbc[:])
                        mx8 = mx8_pool.tile([P, 8], FP32)
                        nc.vector.max(mx8[:], s[:])
                        nc.vector.max_index(
                            idxbig[:, t * 8 : (t + 1) * 8], mx8[:], s[:]
                        )
                        # dequantize gather for this tile's 128 rows
                        nc.gpsimd.indirect_dma_start(
                            out=gth[:, j * D : (j + 1) * D],
                            out_offset=None,
                            in_=e_d.ap(),
                            in_offset=bass.IndirectOffsetOnAxis(
                                ap=idxbig[:, t * 8 : t * 8 + 1], axis=0
                            ),
                        )
                    nc.sync.dma_start(
                        q_d.ap()[:, t0 * D : (t0 + tpc) * D], gth[:, : tpc * D]
                    )
                    nc.scalar.dma_start(
                        out=ind_d.ap()[:, t0 * 8 : (t0 + tpc) * 8],
                        in_=idxbig[:, t0 * 8 : (t0 + tpc) * 8].bitcast(I32),
                    )
                    t0 += tpc

    nc.compile()
    return nc


def kernel(x: np.ndarray, embed: np.ndarray):
    x = np.asarray(x, dtype=np.float32)
    embed = np.ascontiguousarray(np.asarray(embed, dtype=np.float32))
    lead_shape = x.shape[:-1]
    xf = x.reshape(-1, D)
    assert xf.shape == (N_FULL, D) and embed.shape == (K, D)
    xT = np.ascontiguousarray(xf.T)          # [128, 65536], layout-only
    eT = np.ascontiguousarray(embed.T)       # [128, 1024], layout-only

    if "nc" not in _COMPILED:
        _COMPILED["nc"] = _build()
    nc = _COMPILED["nc"]

    in_maps = [
        {
            "xt": np.ascontiguousarray(xT[:, i * N_LOC : (i + 1) * N_LOC]),
            "eT": eT,
            "e": embed,
        }
        for i in range(N_CORES)
    ]
    res = run_bass_kernel_spmd(nc, in_maps, list(range(N_CORES)))

    q = np.empty((N_FULL, D), dtype=np.float32)
    ind = np.empty((N_FULL,), dtype=np.int32)
    for i in range(N_CORES):
        # q device layout [p, t, d] -> row n = t*128 + p
        qi = res.results[i]["q"].reshape(P, TILES, D)
        q[i * N_LOC : (i + 1) * N_LOC] = qi.transpose(1, 0, 2).reshape(N_LOC, D)
        # ind device layout [p, t*8] (lane 0 of each 8-wide slot)
        ia = res.results[i]["ind"].reshape(P, TILES, 8)[:, :, 0]
        ind[i * N_LOC : (i + 1) * N_LOC] = ia.T.reshape(N_LOC).astype(np.int32)

    quantize = q.reshape(*lead_shape, D)
    embed_ind = ind.reshape(*lead_shape)
    return quantize, embed_ind


# revision 31
# speedup vs baseline: 5.7302x; 5.7302x over previous
"""EuclideanCodebook (VQ) forward on 8 Trainium2 NeuronCores.

Data-parallel over the flattened N = batch*frames axis: each core gets an
8192-row slab of x and the full [1024, 128] codebook.

Host-side prep is layout-only (transposes for DMA/matmul efficiency); all
arithmetic runs on device:

  PE:  dist = 2*x·e^T (fp32 matmul into PSUM; the ||x||^2 term of the
       reference is row-constant -> argmax-invariant)
  DVE: adds -||e_k||^2 (broadcast tile built on device at setup) while
       copying PSUM -> SBUF, then Max8 + MaxIndex give the exact fp32
       argmax per row
  GPSIMD: indirect DMA gather embed[idx] -> quantize rows

Device layouts (per core): row n = t*128 + p for tile t in [0,64),
partition p in [0,128).  x arrives transposed ([128 d, 8192 n]) so every
tile's lhsT is a direct SBUF slice; q leaves as [128 p, 64 t, 128 d] and
the host permutes back to row order.
"""

import numpy as np

import concourse.bacc as bacc
import concourse.bass as bass
import concourse.mybir as mybir
import concourse.tile as tile
from concourse.bass_utils import run_bass_kernel_spmd

N_CORES = 8
N_FULL = 65536          # 8 * 8192 rows
N_LOC = N_FULL // N_CORES   # 8192 rows per core
K = 1024                # codebook size
D = 128                 # feature dim
P = 128                 # partitions
TILES = N_LOC // P      # 64 row-tiles per core
CHUNK_SIZES = (4, 12, 16, 16, 12, 4)   # tiles per x/q DMA chunk

FP32 = mybir.dt.float32
U32 = mybir.dt.uint32
I32 = mybir.dt.int32

_COMPILED = {}


def _build(reps=1):
    nc = bacc.Bacc(
        "TRN2", target_bir_lowering=False, debug=False, num_devices=N_CORES
    )
    xt_d = nc.dram_tensor("xt", [P, N_LOC], FP32, kind="ExternalInput")
    eT_d = nc.dram_tensor("eT", [P, K], FP32, kind="ExternalInput")
    e_d = nc.dram_tensor("e", [K, D], FP32, kind="ExternalInput")
    q_d = nc.dram_tensor("q", [P, TILES * D], FP32, kind="ExternalOutput")
    # one 8-wide uint32 slot per row-tile (MaxIndex writes all 8 lanes);
    # host keeps lane 0 of each slot.
    ind_d = nc.dram_tensor("ind", [P, TILES * 8], I32, kind="ExternalOutput")

    with tile.TileContext(nc) as tc:
        with (
            tc.tile_pool(name="const", bufs=1) as const_pool,
            tc.tile_pool(name="xin", bufs=3) as x_pool,
            tc.tile_pool(name="g", bufs=2) as g_pool,
            tc.tile_pool(name="mx8", bufs=2) as mx8_pool,
            tc.tile_pool(name="s", bufs=4) as s_pool,
            tc.tile_pool(name="pdist", bufs=4, space="PSUM") as pdist_pool,
        ):
            # ---- constants / setup ----------------------------------------
            negq = const_pool.tile([P, P], FP32)   # all -0.25
            nc.gpsimd.memset(negq[:], -0.25)
            idxbig = const_pool.tile([P, TILES * 8], U32)

            embT2 = const_pool.tile([P, K], FP32)  # [d, k] = 2*e[k, d]
            eTs = const_pool.tile([P, K], FP32)
            nc.scalar.dma_start(eTs[:], eT_d.ap())
            nc.vector.tensor_scalar_mul(embT2[:], eTs[:], 2.0)
            # sq = (2 e)^2 on ACT; e2negbc[m, k] = -0.25*sum_d sq = -||e_k||^2
            sq = const_pool.tile([P, K], FP32)
            nc.scalar.activation(
                sq[:], embT2[:], mybir.ActivationFunctionType.Square
            )
            e2bc_ps = pdist_pool.tile([P, K], FP32, space="PSUM", tag="pd")
            for h in range(2):
                sl = slice(h * 512, (h + 1) * 512)
                nc.tensor.matmul(
                    out=e2bc_ps[:, sl],
                    lhsT=negq[:],
                    rhs=sq[:, sl],
                    start=True,
                    stop=True,
                )
            e2negbc = const_pool.tile([P, K], FP32)
            nc.vector.tensor_copy(e2negbc[:], e2bc_ps[:])

            # ---- main loop (reps>1 only for steady-state timing probes) ---
            for _rep in range(reps):
                t0 = 0
                for tpc in CHUNK_SIZES:
                    xc = x_pool.tile([P, 16 * P], FP32, tag="xc")
                    nc.sync.dma_start(
                        xc[:, : tpc * P], xt_d.ap()[:, t0 * P : (t0 + tpc) * P]
                    )
                    gth = g_pool.tile([P, 16 * D], FP32, tag="gth")
                    for j in range(tpc):
                        t = t0 + j
                        dist_ps = pdist_pool.tile(
                            [P, K], FP32, space="PSUM", tag="pd"
                        )
                        # PE computes raw 2x·e; DVE folds in -||e_k||^2
                        # while copying to SBUF, then scans there.
                        for h in range(2):
                            sl = slice(h * 512, (h + 1) * 512)
                            nc.tensor.matmul(
                                out=dist_ps[:, sl],
                                lhsT=xc[:, j * P : (j + 1) * P],
                                rhs=embT2[:, sl],
                                start=True,
                                stop=True,
                            )
                        s = s_pool.tile([P, K], FP32, tag="s")
                        nc.vector.tensor_add(s[:], dist_ps[:], e2negbc[:])
                        mx8 = mx8_pool.tile([P, 8], FP32)
                        nc.vector.max(mx8[:], s[:])
                        nc.vector.max_index(
                            idxbig[:, t * 8 : (t + 1) * 8], mx8[:], s[:]
                        )
                        # dequantize gather for this tile's 128 rows
                        nc.gpsimd.indirect_dma_start(
                            out=gth[:, j * D : (j + 1) * D],
                            out_offset=None,
                            in_=e_d.ap(),
                            in_offset=bass.IndirectOffsetOnAxis(
                                ap=idxbig[:, t * 8 : t * 8 + 1], axis=0
                            ),
                        )
                    # q writeback on the scalar HWDGE ring so it doesn't
                    # queue behind the next x chunk load on the sync ring
                    nc.scalar.dma_start(
                        q_d.ap()[:, t0 * D : (t0 + tpc) * D], gth[:, : tpc * D]
                    )
                    nc.scalar.dma_start(
                        out=ind_d.ap()[:, t0 * 8 : (t0 + tpc) * 8],
                        in_=idxbig[:, t0 * 8 : (t0 + tpc) * 8].bitcast(I32),
                    )
                    t0 += tpc

    nc.compile()
    return nc


def kernel(x: np.ndarray, embed: np.ndarray):
    x = np.asarray(x, dtype=np.float32)
    embed = np.ascontiguousarray(np.asarray(embed, dtype=np.float32))
    lead_shape = x.shape[:-1]
    xf = x.reshape(-1, D)
    assert xf.shape == (N_FULL, D) and embed.shape == (K, D)
    xT = np.ascontiguousarray(xf.T)          # [128, 65536], layout-only
    eT = np.ascontiguousarray(embed.T)       # [128, 1024], layout-only

    if "nc" not in _COMPILED:
        _COMPILED["nc"] = _build()
    nc = _COMPILED["nc"]

    in_maps = [
        {
            "xt": np.ascontiguousarray(xT[:, i * N_LOC : (i + 1) * N_LOC]),
            "eT": eT,
            "e": embed,
        }
        for i in range(N_CORES)
    ]
    res = run_bass_kernel_spmd(nc, in_maps, list(range(N_CORES)))

    q = np.empty((N_FULL, D), dtype=np.float32)
    ind = np.empty((N_FULL,), dtype=np.int32)
    for i in range(N_CORES):
        # q device layout [p, t, d] -> row n = t*128 + p
        qi = res.results[i]["q"].reshape(P, TILES, D)
        q[i * N_LOC : (i + 1) * N_LOC] = qi.transpose(1, 0, 2).reshape(N_LOC, D)
        # ind device layout [p, t*8] (lane 0 of each 8-wide slot)
        ia = res.results[i]["ind"].reshape(P, TILES, 8)[:, :, 0]
        ind[i * N_LOC : (i + 1) * N_LOC] = ia.T.reshape(N_LOC).astype(np.int32)

    quantize = q.reshape(*lead_shape, D)
    embed_ind = ind.reshape(*lead_shape)
    return quantize, embed_ind
